# revision 13
# baseline (speedup 1.0000x reference)
"""Causal self-attention Trainium2 kernel.

Problem: y = CausalSelfAttention(x) with B=4, T=2048, C=1024, H=16 heads,
head_dim D=64, qkv split order (k, q, v), softmax scale C**-0.5.

Sharding (8 cores): core = 2*b + g  -> batch b in 0..3, head-group g in 0..1
(8 heads per group).  Each core computes, for its batch and its 8 heads:
  qkv partial matmuls, causal attention, and the partial output projection
  y_partial = att_out @ W_proj[rows of this head group].
The host sums the two partial projections per batch (row-parallel tensor
parallelism reduced on host during unsharding).

Device layout notes (per core):
  xT    [128, 8, 2048]  x^T (C on partitions), loaded via DMA transpose (bf16)
  kqT   [128, 8, 2048]  (x @ W_kq)^T : blocks 0-3 = k-channels, 4-7 = q-channels
                         head h: 64*(h%2) partition offset, block h//2 (+4 for q)
  v_aug [128, 16, 520]  v in natural layout, 65 cols/head = [v(64) | ones(1)]
  S^T   [k partitions, q free] -> exp on ACT (scale 1/32, fp32 PSUM -> bf16)
        full 128k-blocks computed in pairs (one 2-bank PSUM tile per pair);
        diagonal-band blocks col-sliced to the valid columns, triangular mask
        zeroed via gpsimd affine_select on the leading 128 columns.
  AV:   out^T[65, q] = [V|1]^T @ P^T accumulated over k tiles; row 64 = softmax
        denominator.  reciprocal (DVE) -> shift to partition 0 (DMA) ->
        partition_broadcast (gpsimd) -> multiply (DVE) -> place into att
        (SBUF->SBUF DMA, handles the odd-head partition offset).
  proj: y^T[1024, 2048] = W_proj_g(lhsT) @ att^T, streamed to HBM in fp32.
"""

import numpy as np
import ml_dtypes

B, T, C, H = 4, 2048, 1024, 16
D = C // H          # 64
HPC = H // 2        # 8 heads per core
CG = C // 2         # 512 channels per head group
P = 128

_compiled = {}


def _build(t=T):
    import concourse.bacc as bacc
    import concourse.tile as tile
    import concourse.mybir as mybir

    f32 = mybir.dt.float32
    bf16 = mybir.dt.bfloat16
    Exp = mybir.ActivationFunctionType.Exp

    KT = C // P            # 8 contraction tiles over C
    MB = (2 * CG) // P     # 8 kq channel blocks (0-3 k, 4-7 q)
    TT = t // P            # token tiles of 128
    QC = t // 512          # q chunks of 512
    VB = CG // P           # 4 v/att channel blocks
    SCALE = float(C) ** -0.5

    nc = bacc.Bacc("TRN2", target_bir_lowering=False, debug=False,
                   num_devices=8)

    x_d = nc.dram_tensor("x", [t, C], bf16, kind="ExternalInput")
    wkq_d = nc.dram_tensor("wkq", [C, 2 * CG], bf16, kind="ExternalInput")
    wv_d = nc.dram_tensor("wv", [C, CG], bf16, kind="ExternalInput")
    wp_d = nc.dram_tensor("wp", [CG, C], bf16, kind="ExternalInput")
    y_d = nc.dram_tensor("y", [C, t], f32, kind="ExternalOutput")

    with tile.TileContext(nc) as tc:
        with (
            tc.tile_pool(name="persist", bufs=1) as persist,
            tc.tile_pool(name="psA", bufs=2, space="PSUM") as psA,
            tc.tile_pool(name="avP", bufs=2, space="PSUM") as avP,
            tc.tile_pool(name="ptP", bufs=14) as ptP,
            tc.tile_pool(name="ptdP", bufs=10) as ptdP,
            tc.tile_pool(name="rcP", bufs=2) as rcP,
            tc.tile_pool(name="rbP", bufs=2) as rbP,
            tc.tile_pool(name="atP", bufs=3) as atP,
            tc.tile_pool(name="yP", bufs=3) as yP,
        ):
            xT = persist.tile([P, KT, t], bf16)
            wkq_sb = persist.tile([P, KT, 2 * CG], bf16)
            wv_sb = persist.tile([P, KT, CG], bf16)
            wp_sb = persist.tile([P, VB, C], bf16)
            kqT = persist.tile([P, MB, t], bf16)
            v_aug = persist.tile([P, TT, HPC * (D + 1)], bf16)
            att = persist.tile([P, VB, t], bf16)

            # ---- loads ----
            for ct in range(KT):
                nc.sync.dma_start(xT[:, ct, :], x_d[:, ct * P:(ct + 1) * P],
                                  transpose=True)
            nc.sync.dma_start(
                wkq_sb, wkq_d.ap().rearrange("(kt p) m -> p kt m", p=P))
            nc.sync.dma_start(
                wv_sb, wv_d.ap().rearrange("(kt p) m -> p kt m", p=P))
            nc.sync.dma_start(
                wp_sb, wp_d.ap().rearrange("(kt p) m -> p kt m", p=P))
            nc.vector.memset(v_aug, 1.0)

            def chunk_pairs(n):
                return [list(range(i, min(i + 2, n))) for i in range(0, n, 2)]

            def emit_kq_block(mb):
                for grp in chunk_pairs(QC):
                    ps = psA.tile([P, 2, 512], f32, name="ps", tag="st")
                    for kt in range(KT):
                        for u, c in enumerate(grp):
                            nc.tensor.matmul(
                                ps[:, u, :],
                                lhsT=wkq_sb[:, kt, mb * P:(mb + 1) * P],
                                rhs=xT[:, kt, c * 512:(c + 1) * 512],
                                start=(kt == 0), stop=(kt == KT - 1),
                                skip_group_check=True)
                    nc.vector.tensor_copy(
                        kqT[:, mb, grp[0] * 512:(grp[-1] + 1) * 512],
                        ps[:, 0:len(grp), :].rearrange("p u n -> p (u n)"))

            def emit_v():
                for grp in chunk_pairs(TT):
                    ps = psA.tile([P, 2, CG], f32, name="psv", tag="st")
                    for kt in range(KT):
                        for u, tt in enumerate(grp):
                            nc.tensor.matmul(
                                ps[:, u, :],
                                lhsT=xT[:, kt, tt * P:(tt + 1) * P],
                                rhs=wv_sb[:, kt, :],
                                start=(kt == 0), stop=(kt == KT - 1),
                                skip_group_check=True)
                    nc.vector.tensor_copy(
                        v_aug[:, grp[0]:grp[-1] + 1, :].rearrange(
                            "p u (h e) -> p u h e", e=D + 1)[:, :, :, 0:D],
                        ps[:, 0:len(grp), :].rearrange(
                            "p u (h d) -> p u h d", d=D))

            def emit_attn(hp):
                for c in range(QC):
                    nfull = 4 * c
                    avp = [avP.tile([D + 1, 512], f32, name=f"avp{hi}",
                                    tag=f"avp{hi}")
                           for hi in range(2)]
                    work = [[], []]
                    for jp in range(nfull // 2):
                        for hi in range(2):
                            lo = D * hi
                            st = psA.tile([P, 2, 512], f32, name="st",
                                          tag="st")
                            for u in range(2):
                                j = 2 * jp + u
                                nc.tensor.matmul(
                                    st[:, u, :],
                                    lhsT=kqT[lo:lo + D, hp,
                                             j * P:(j + 1) * P],
                                    rhs=kqT[lo:lo + D, 4 + hp,
                                            c * 512:(c + 1) * 512],
                                    start=True, stop=True,
                                    skip_group_check=True)
                            pt = ptP.tile([P, 2, 512], bf16, name="pt",
                                          tag="pt")
                            nc.scalar.activation(pt, st, Exp, scale=SCALE)
                            for u in range(2):
                                work[hi].append((pt[:, u, :], 2 * jp + u, 0))
                    for dj in range(4):
                        j = nfull + dj
                        off = P * dj
                        w = 512 - off
                        for hi in range(2):
                            lo = D * hi
                            st = psA.tile([P, 2, 512], f32, name="std",
                                          tag="st")
                            nc.tensor.matmul(
                                st[:, 0, 0:w],
                                lhsT=kqT[lo:lo + D, hp, j * P:(j + 1) * P],
                                rhs=kqT[lo:lo + D, 4 + hp,
                                        c * 512 + off:(c + 1) * 512],
                                start=True, stop=True,
                                skip_group_check=True)
                            pt = ptdP.tile([P, 512], bf16, name="ptd",
                                           tag="ptd")
                            nc.scalar.activation(pt[:, 0:w], st[:, 0, 0:w],
                                                 Exp, scale=SCALE)
                            nc.gpsimd.affine_select(
                                pt[:, 0:P], pt[:, 0:P],
                                pattern=[[1, P]],
                                compare_op=mybir.AluOpType.is_ge,
                                fill=0.0, base=0, channel_multiplier=-1)
                            work[hi].append((pt[:, 0:w], j, off))
                    for hi in range(2):
                        h = 2 * hp + hi
                        n = len(work[hi])
                        for idx, (pap, j, off) in enumerate(work[hi]):
                            out_ap = avp[hi][:, off:512] if off else avp[hi]
                            nc.tensor.matmul(
                                out_ap,
                                lhsT=v_aug[:, j,
                                           h * (D + 1):(h + 1) * (D + 1)],
                                rhs=pap,
                                start=(idx == 0), stop=(idx == n - 1),
                                skip_group_check=True)
                        rc = rcP.tile([D + 1, 512], f32)
                        nc.vector.reciprocal(rc[D:D + 1, :],
                                             avp[hi][D:D + 1, :])
                        # partition_broadcast only reads physical partition 0;
                        # DMA-shift the reciprocal row there first.
                        rc0 = rcP.tile([1, 512], f32, name="rc0", tag="rc0")
                        nc.sync.dma_start(rc0, rc[D:D + 1, :])
                        rb = rbP.tile([D, 512], f32)
                        nc.gpsimd.partition_broadcast(rb, rc0[0:1, :],
                                                      channels=D)
                        at = atP.tile([D, 512], bf16)
                        nc.vector.tensor_mul(at, avp[hi][0:D, :], rb)
                        nc.sync.dma_start(
                            att[D * hi:D * (hi + 1), hp,
                                c * 512:(c + 1) * 512],
                            at)

            # qkv for head-pair hp needs kq blocks hp (k) and 4+hp (q)
            emit_kq_block(0)
            emit_kq_block(4)
            emit_v()
            emit_attn(0)
            for hp in range(1, 4):
                emit_kq_block(hp)
                emit_kq_block(4 + hp)
                emit_attn(hp)

            # ---- projection: y^T = W_proj_g(lhsT) @ att^T ----
            for mb in range(C // P):
                for grp in chunk_pairs(QC):
                    ps = psA.tile([P, 2, 512], f32, name="psp", tag="st")
                    for kt in range(VB):
                        for u, c in enumerate(grp):
                            nc.tensor.matmul(
                                ps[:, u, :],
                                lhsT=wp_sb[:, kt, mb * P:(mb + 1) * P],
                                rhs=att[:, kt, c * 512:(c + 1) * 512],
                                start=(kt == 0), stop=(kt == VB - 1),
                                skip_group_check=True)
                    yt = yP.tile([P, 2, 512], f32)
                    nc.vector.tensor_copy(yt[:, 0:len(grp), :],
                                          ps[:, 0:len(grp), :])
                    nc.sync.dma_start(
                        y_d[mb * P:(mb + 1) * P,
                            grp[0] * 512:(grp[-1] + 1) * 512],
                        yt[:, 0:len(grp), :].rearrange("p u n -> p (u n)"))

    nc.compile()
    return nc


def _get_compiled(t=T):
    if t not in _compiled:
        _compiled[t] = _build(t)
    return _compiled[t]


def make_in_maps(x, W_qkv, W_proj):
    bf = ml_dtypes.bfloat16
    in_maps = []
    for core in range(8):
        b, g = core // 2, core % 2
        in_maps.append({
            "x": np.ascontiguousarray(x[b]).astype(bf),
            "wkq": np.concatenate(
                [W_qkv[:, g * CG:(g + 1) * CG],
                 W_qkv[:, C + g * CG:C + (g + 1) * CG]], axis=1).astype(bf),
            "wv": np.ascontiguousarray(
                W_qkv[:, 2 * C + g * CG:2 * C + (g + 1) * CG]).astype(bf),
            "wp": np.ascontiguousarray(
                W_proj[g * CG:(g + 1) * CG, :]).astype(bf),
        })
    return in_maps


def kernel(x, W_qkv, W_proj, _trace=False):
    import concourse.bass_utils as bass_utils

    nc = _get_compiled()
    in_maps = make_in_maps(x, W_qkv, W_proj)
    res = bass_utils.run_bass_kernel_spmd(
        nc, in_maps, core_ids=list(range(8)), trace=_trace)
    y = np.zeros((B, T, C), np.float32)
    for core in range(8):
        y[core // 2] += res.results[core]["y"].T
    if _trace:
        kernel.last_results = res
    return y


# revision 38
# speedup vs baseline: 312.8840x; 312.8840x over previous
"""Causal self-attention Trainium2 kernel.

Problem: y = CausalSelfAttention(x) with B=4, T=2048, C=1024, H=16 heads,
head_dim D=64, qkv split order (k, q, v), softmax scale C**-0.5.

Sharding (8 cores): core = 2*b + g  -> batch b in 0..3, head-group g in 0..1
(8 heads per group).  Each core computes, for its batch and its 8 heads:
  qkv partial matmuls, causal attention, and the partial output projection
  y_partial = att_out @ W_proj[rows of this head group].
The host sums the two partial projections per batch (row-parallel tensor
parallelism reduced on host during unsharding).

Device layout notes (per core):
  xT    [128, 8, 2048]  x^T (C on partitions), loaded via DMA transpose (bf16)
  kqT   [128, 8, 2048]  (x @ W_kq)^T : blocks 0-3 = k-channels, 4-7 = q-channels
                         head h: 64*(h%2) partition offset, block h//2 (+4 for q)
  v_aug [128, 16, 520]  v in natural layout, 65 cols/head = [v(64) | ones(1)]
  S^T   [k partitions, q free] -> exp on ACT (scale 1/32, fp32 PSUM -> bf16)
        full 128k-blocks computed in per-head pairs (2-bank PSUM tile per
        pair); diagonal-band blocks col-sliced to valid columns and paired
        across the two concurrently-processed heads; the triangular mask is
        zeroed via gpsimd affine_select on the leading 128 columns.
  AV:   out^T[65, q] = [V|1]^T @ P^T accumulated over k tiles; row 64 = softmax
        denominator.  reciprocal (DVE) -> shift to partition 0 (DMA) ->
        partition_broadcast (gpsimd) -> multiply (DVE) -> place into att
        (SBUF->SBUF DMA, handles the odd-head partition offset).
  proj: y^T[1024, 2048] = W_proj_g(lhsT) @ att^T, streamed to HBM in fp32.

Scheduling: qkv for head-pair hp+1 is emitted interleaved with the attention
chunks of head-pair hp (separate PSUM tag) so the PE fills ACT-bound exp
windows with qkv matmuls.
"""

import numpy as np
import ml_dtypes

B, T, C, H = 4, 2048, 1024, 16
D = C // H          # 64
HPC = H // 2        # 8 heads per core
CG = C // 2         # 512 channels per head group
P = 128

_compiled = {}


def _build(t=T):
    import concourse.bacc as bacc
    import concourse.tile as tile
    import concourse.mybir as mybir

    f32 = mybir.dt.float32
    bf16 = mybir.dt.bfloat16
    Exp = mybir.ActivationFunctionType.Exp

    KT = C // P            # 8 contraction tiles over C
    MB = (2 * CG) // P     # 8 kq channel blocks (0-3 k, 4-7 q)
    TT = t // P            # token tiles of 128
    QC = t // 512          # q chunks of 512
    VB = CG // P           # 4 v/att channel blocks
    SCALE = float(C) ** -0.5

    nc = bacc.Bacc("TRN2", target_bir_lowering=False, debug=False,
                   num_devices=8)

    x_d = nc.dram_tensor("x", [t, C], bf16, kind="ExternalInput")
    wkq_d = nc.dram_tensor("wkq", [C, 2 * CG], bf16, kind="ExternalInput")
    wv_d = nc.dram_tensor("wv", [C, CG], bf16, kind="ExternalInput")
    wp_d = nc.dram_tensor("wp", [CG, C], bf16, kind="ExternalInput")
    y_d = nc.dram_tensor("y", [C, t], f32, kind="ExternalOutput")

    with tile.TileContext(nc) as tc:
        with (
            tc.tile_pool(name="persist", bufs=1) as persist,
            tc.tile_pool(name="psA", bufs=2, space="PSUM") as psA,
            tc.tile_pool(name="avP", bufs=1, space="PSUM") as avP,
            tc.tile_pool(name="ptP", bufs=16) as ptP,
            tc.tile_pool(name="ptdP", bufs=8) as ptdP,
            tc.tile_pool(name="rcP", bufs=2) as rcP,
            tc.tile_pool(name="rbP", bufs=2) as rbP,
            tc.tile_pool(name="atP", bufs=3) as atP,
            tc.tile_pool(name="yP", bufs=3) as yP,
        ):
            xT = persist.tile([P, KT, t], bf16)
            wkq_sb = persist.tile([P, KT, 2 * CG], bf16)
            wv_sb = persist.tile([P, KT, CG], bf16)
            wp_sb = persist.tile([P, VB, C], bf16)
            kqT = persist.tile([P, MB, t], bf16)
            v_aug = persist.tile([P, TT, HPC * (D + 1)], bf16)
            att = persist.tile([P, VB, t], bf16)

            dma_engs = [nc.sync, nc.sync]

            # PE warm-up: dependency-free matmuls run during the input-DMA
            # window so the HAM clock gate is at 8/8 when real work starts.
            wu_a = persist.tile([P, P], bf16)
            wu_b = persist.tile([P, 512], bf16)
            nc.vector.memset(wu_a, 0.0)
            nc.vector.memset(wu_b, 0.0)
            for _ in range(24):
                wps = psA.tile([P, 512], f32, name="wups", tag="qp", bufs=1)
                nc.tensor.matmul(wps, lhsT=wu_a, rhs=wu_b,
                                 start=True, stop=True,
                                 skip_group_check=True)

            # ---- loads: split across both HWDGE queues ----
            for ct in range(KT):
                dma_engs[ct % 2].dma_start(
                    xT[:, ct, :], x_d[:, ct * P:(ct + 1) * P], transpose=True)
            wkq_r = wkq_d.ap().rearrange("(kt p) m -> p kt m", p=P)
            wv_r = wv_d.ap().rearrange("(kt p) m -> p kt m", p=P)
            wp_r = wp_d.ap().rearrange("(kt p) m -> p kt m", p=P)
            for kt in range(KT):
                dma_engs[(kt + 1) % 2].dma_start(
                    wkq_sb[:, kt, :], wkq_r[:, kt, :])
                dma_engs[kt % 2].dma_start(wv_sb[:, kt, :], wv_r[:, kt, :])
            for kt in range(VB):
                dma_engs[kt % 2].dma_start(wp_sb[:, kt, :], wp_r[:, kt, :])
            nc.vector.memset(v_aug, 1.0)

            def chunk_pairs(n):
                return [list(range(i, min(i + 2, n))) for i in range(0, n, 2)]

            # one qkv "unit" = one PSUM accumulation group; "st"-tag units
            # use a 2-chunk (2-bank) tile, "qp"-tag units a 1-chunk tile
            def emit_kq_unit(mb, grp, tag):
                nu = 2 if tag == "st" else 1
                grp = grp if tag == "st" else grp[:1]
                ps = psA.tile([P, nu, 512], f32, name="ps", tag=tag,
                              bufs=(2 if tag == "st" else 1))
                for kt in range(KT):
                    for u, c in enumerate(grp):
                        nc.tensor.matmul(
                            ps[:, u, :],
                            lhsT=wkq_sb[:, kt, mb * P:(mb + 1) * P],
                            rhs=xT[:, kt, c * 512:(c + 1) * 512],
                            start=(kt == 0), stop=(kt == KT - 1),
                            skip_group_check=True)
                nc.vector.tensor_copy(
                    kqT[:, mb, grp[0] * 512:(grp[-1] + 1) * 512],
                    ps[:, 0:len(grp), :].rearrange("p u n -> p (u n)"))

            def emit_v_unit(grp, tag):
                nu = 2 if tag == "st" else 1
                grp = grp if tag == "st" else grp[:1]
                ps = psA.tile([P, nu, CG], f32, name="psv", tag=tag,
                              bufs=(2 if tag == "st" else 1))
                for kt in range(KT):
                    for u, tt in enumerate(grp):
                        nc.tensor.matmul(
                            ps[:, u, :],
                            lhsT=xT[:, kt, tt * P:(tt + 1) * P],
                            rhs=wv_sb[:, kt, :],
                            start=(kt == 0), stop=(kt == KT - 1),
                            skip_group_check=True)
                nc.vector.tensor_copy(
                    v_aug[:, grp[0]:grp[-1] + 1, :].rearrange(
                        "p u (h e) -> p u h e", e=D + 1)[:, :, :, 0:D],
                    ps[:, 0:len(grp), :].rearrange(
                        "p u (h d) -> p u h d", d=D))

            def emit_attn_chunk(hp, c):
                nfull = 4 * c
                avp = [avP.tile([D + 1, 512], f32, name=f"avp{hi}",
                                tag="avp", bufs=3)
                       for hi in range(2)]
                work = [[], []]
                for j in range(nfull):
                    # both heads' S^T for k-tile j in one 2-bank tile: the
                    # two matmuls are PE-adjacent with different row groups
                    # (rows 0:64 vs 64:128) so the systolic array overlaps
                    # them; one exp covers both heads
                    st = psA.tile([P, 2, 512], f32, name="st", tag="st")
                    for hi in range(2):
                        lo = D * hi
                        nc.tensor.matmul(
                            st[:, hi, :],
                            lhsT=kqT[lo:lo + D, hp, j * P:(j + 1) * P],
                            rhs=kqT[lo:lo + D, 4 + hp,
                                    c * 512:(c + 1) * 512],
                            start=True, stop=True,
                            skip_group_check=True)
                    pt = ptP.tile([P, 2, 512], bf16, name="pt", tag="pt")
                    nc.scalar.activation(pt, st, Exp, scale=SCALE)
                    for hi in range(2):
                        work[hi].append((pt[:, hi, :], j, 0))
                for dj in range(4):
                    j = nfull + dj
                    off = P * dj
                    w = 512 - off
                    st = psA.tile([P, 2, 512], f32, name="std", tag="st")
                    for hi in range(2):
                        lo = D * hi
                        nc.tensor.matmul(
                            st[:, hi, 0:w],
                            lhsT=kqT[lo:lo + D, hp, j * P:(j + 1) * P],
                            rhs=kqT[lo:lo + D, 4 + hp,
                                    c * 512 + off:(c + 1) * 512],
                            start=True, stop=True,
                            skip_group_check=True)
                    pt = ptdP.tile([P, 2, 512], bf16, name="ptd", tag="ptd")
                    nc.scalar.activation(pt[:, :, 0:w], st[:, :, 0:w],
                                         Exp, scale=SCALE)
                    nc.gpsimd.affine_select(
                        pt[:, :, 0:P], pt[:, :, 0:P],
                        pattern=[[0, 2], [1, P]],
                        compare_op=mybir.AluOpType.is_ge,
                        fill=0.0, base=0, channel_multiplier=-1)
                    for hi in range(2):
                        work[hi].append((pt[:, hi, 0:w], j, off))
                for hi in range(2):
                    h = 2 * hp + hi
                    n = len(work[hi])
                    for idx, (pap, j, off) in enumerate(work[hi]):
                        out_ap = avp[hi][:, off:512] if off else avp[hi]
                        nc.tensor.matmul(
                            out_ap,
                            lhsT=v_aug[:, j, h * (D + 1):(h + 1) * (D + 1)],
                            rhs=pap,
                            start=(idx == 0), stop=(idx == n - 1),
                            skip_group_check=True)
                    rc = rcP.tile([D + 1, 512], f32)
                    nc.vector.reciprocal(rc[D:D + 1, :], avp[hi][D:D + 1, :])
                    # partition_broadcast only reads physical partition 0;
                    # DMA-shift the reciprocal row there first.
                    rc0 = rcP.tile([1, 512], f32, name="rc0", tag="rc0")
                    nc.sync.dma_start(rc0, rc[D:D + 1, :])
                    rb = rbP.tile([D, 512], f32)
                    nc.gpsimd.partition_broadcast(rb, rc0[0:1, :], channels=D)
                    at = atP.tile([D, 512], bf16)
                    nc.vector.tensor_mul(at, avp[hi][0:D, :], rb)
                    nc.sync.dma_start(
                        att[D * hi:D * (hi + 1), hp, c * 512:(c + 1) * 512],
                        at)

            # ---- startup: just enough for attn(0, 0..1), alternate tags ----
            cps = chunk_pairs(QC)
            vps = chunk_pairs(TT)
            startup = [("kq", 0, cps[0]), ("kq", 4, cps[0])]
            startup += [("v", None, g) for g in vps[0:2]]
            for i, (kind, mb, grp) in enumerate(startup):
                if kind == "kq":
                    emit_kq_unit(mb, grp, "st")
                else:
                    emit_v_unit(grp, "st")

            # Remaining qkv/v units (single-chunk, 1-bank "qp" tiles),
            # emitted as PE filler between attention chunks.  Tile discovers
            # dependencies from TRACE order, so a producer MUST be emitted
            # before its first consumer chunk; each fill carries the global
            # chunk index it is first needed by.
            def cdiv(a, b):
                return -(-a // b)

            fills = []
            for tt in range(4, TT):
                # attn(0, c) AV reads v tiles tt <= 4c+3
                fills.append((max(0, cdiv(tt - 3, 4)), ("v", None, [tt])))
            for hp in range(4):
                for ck in range(QC):
                    if hp == 0 and ck in (0, 1):
                        continue
                    # k-side: attn(hp, c) reads j-tiles <= 4c+3 of block hp
                    fills.append((4 * hp + ck, ("kq", hp, [ck])))
                    # q-side: attn(hp, c) reads q chunk c of block 4+hp
                    fills.append((4 * hp + ck, ("kq", 4 + hp, [ck])))
            fills.sort(key=lambda f: f[0])

            # ---- attention with interleaved filler units ----
            nchunks = 4 * QC
            emitted = 0

            def emit_fills(upto):
                nonlocal emitted
                while emitted < min(upto, len(fills)):
                    _, (kind, mb, grp) = fills[emitted]
                    if kind == "kq":
                        emit_kq_unit(mb, grp, "qp")
                    else:
                        emit_v_unit(grp, "qp")
                    emitted += 1

            for hp in range(4):
                for c in range(QC):
                    ci = hp * QC + c
                    # everything this chunk reads must already be emitted
                    while emitted < len(fills) and fills[emitted][0] <= ci:
                        emit_fills(emitted + 1)
                    emit_attn_chunk(hp, c)
                    emit_fills(((ci + 4) * len(fills)) // nchunks)
            emit_fills(len(fills))

            # ---- projection: y^T = W_proj_g(lhsT) @ att^T ----
            # gi-major so the first-half chunks (ready before the final
            # attention chunks finish) are emitted first
            for gi, grp in enumerate(chunk_pairs(QC)):
                for mb in range(C // P):
                    ps = psA.tile([P, 2, 512], f32, name="psp", tag="st",
                                  bufs=2)
                    for kt in range(VB):
                        for u, c in enumerate(grp):
                            nc.tensor.matmul(
                                ps[:, u, :],
                                lhsT=wp_sb[:, kt, mb * P:(mb + 1) * P],
                                rhs=att[:, kt, c * 512:(c + 1) * 512],
                                start=(kt == 0), stop=(kt == VB - 1),
                                skip_group_check=True)
                    yt = yP.tile([P, 2, 512], f32)
                    nc.vector.tensor_copy(yt[:, 0:len(grp), :],
                                          ps[:, 0:len(grp), :])
                    nc.sync.dma_start(
                        y_d[mb * P:(mb + 1) * P,
                            grp[0] * 512:(grp[-1] + 1) * 512],
                        yt[:, 0:len(grp), :].rearrange("p u n -> p (u n)"))

    nc.compile()
    return nc


def _get_compiled(t=T):
    if t not in _compiled:
        _compiled[t] = _build(t)
    return _compiled[t]


def make_in_maps(x, W_qkv, W_proj):
    bf = ml_dtypes.bfloat16
    in_maps = []
    for core in range(8):
        b, g = core // 2, core % 2
        in_maps.append({
            "x": np.ascontiguousarray(x[b]).astype(bf),
            "wkq": np.concatenate(
                [W_qkv[:, g * CG:(g + 1) * CG],
                 W_qkv[:, C + g * CG:C + (g + 1) * CG]], axis=1).astype(bf),
            "wv": np.ascontiguousarray(
                W_qkv[:, 2 * C + g * CG:2 * C + (g + 1) * CG]).astype(bf),
            "wp": np.ascontiguousarray(
                W_proj[g * CG:(g + 1) * CG, :]).astype(bf),
        })
    return in_maps


def _run_axon_nodonate(nc, in_maps, n_cores=8):
    """Execute via PJRT/shard_map WITHOUT output-buffer donation.

    bass2jax.run_bass_via_pjrt donates the zero output operands; under the
    axon transport that donation intermittently corrupts multi-core results.
    This kernel writes every element of its output, so donation is not
    needed for correctness -- pass non-donated zero operands instead.
    """
    import jax
    from jax.sharding import Mesh, PartitionSpec
    from jax.experimental.shard_map import shard_map
    import concourse.mybir as mybir
    from concourse.bass2jax import _bass_exec_p, install_neuronx_cc_hook

    install_neuronx_cc_hook()
    in_names, out_names, out_avals = [], [], []
    for alloc in nc.m.functions[0].allocations:
        if not isinstance(alloc, mybir.MemoryLocationSet):
            continue
        name = alloc.memorylocations[0].name
        if alloc.kind == "ExternalInput":
            in_names.append(name)
        elif alloc.kind == "ExternalOutput":
            out_names.append(name)
            out_avals.append(jax.core.ShapedArray(
                tuple(alloc.tensor_shape), mybir.dt.np(alloc.dtype)))
    n_params = len(in_names)
    all_names = in_names + out_names
    pid_name = nc.partition_id_tensor.name if nc.partition_id_tensor else None

    def _body(*args):
        return tuple(_bass_exec_p.bind(
            *args,
            out_avals=tuple(out_avals),
            in_names=tuple(all_names),
            out_names=tuple(out_names),
            lowering_input_output_aliases=(),
            sim_require_finite=True,
            sim_require_nnan=True,
            nc=nc,
        ))

    devices = jax.devices()[:n_cores]
    mesh = Mesh(np.asarray(devices), ("core",))
    fn = jax.jit(
        shard_map(_body, mesh=mesh,
                  in_specs=(PartitionSpec("core"),) * (n_params + len(out_names)),
                  out_specs=(PartitionSpec("core"),) * len(out_names),
                  check_rep=False),
        keep_unused=True)
    concat_in = [
        np.concatenate([
            np.asarray(in_maps[c].get(
                nm, np.array([[c]], dtype=np.uint32) if nm == pid_name
                else None))
            for c in range(n_cores)], 0)
        for nm in in_names
    ]
    concat_zeros = [
        np.zeros((n_cores * a.shape[0], *a.shape[1:]), a.dtype)
        for a in out_avals
    ]
    out = fn(*concat_in, *concat_zeros)
    return [
        {nm: np.asarray(out[i]).reshape(n_cores, *out_avals[i].shape)[c]
         for i, nm in enumerate(out_names)}
        for c in range(n_cores)
    ]


def kernel(x, W_qkv, W_proj, _trace=False):
    from concourse._compat import axon_active

    nc = _get_compiled()
    in_maps = make_in_maps(x, W_qkv, W_proj)
    if axon_active():
        results = _run_axon_nodonate(nc, in_maps)
    else:
        import concourse.bass_utils as bass_utils
        res = bass_utils.run_bass_kernel_spmd(
            nc, in_maps, core_ids=list(range(8)), trace=_trace)
        if _trace:
            kernel.last_results = res
        results = res.results
    y = np.zeros((B, T, C), np.float32)
    for core in range(8):
        y[core // 2] += results[core]["y"].T
    return y
